# revision 2
# baseline (speedup 1.0000x reference)
"""GCN layer relu((A_hat @ x) @ W + b) on 8 TRN2 NeuronCores (Bass/Tile).

Self-contained: kernel(**inputs) takes FULL inputs, returns FULL output.

Strategy (dst-sharded SPMD, one program on 8 cores), gather-bound design:
  - nodes (rows of x / output) split into 8 contiguous shards of 12500;
    per core 98 dst windows of 128.
  - edges (self-loops excluded) regioned by (src chunk of 25000, window);
    regions padded to 128-slot blocks using the max count across cores so
    one SPMD program fits all. Pad slots use SPREAD dummy indices (same-row
    hotspots serialize DRAM banks) and are killed by norm=0.
  - src features pulled from a bf16 copy of x with TRANSPOSE-mode SWDGE
    indirect DMA (measured ~25% faster than slot-major mode), calls of <=64
    blocks; per call a batched stage: TensorE transpose-back of each block
    (via identity) into PSUM, ScalarE copy into a per-call bf16 mega-tile.
  - per block: DVE builds the scaled one-hot st[e,d] =
    (cmp==rel[e])*norm[e] in ONE fused tensor_scalar op (bf16 out);
    TensorE accumulates psum[d,f] += st.T @ g_block (bf16, 1 cyc/row).
  - self-loops bypass the gather: agg_w += dinv^2[d]*xshard[d] via a
    ScalarE scale of a sequentially-loaded shard at first-chunk flush.
  - phase 2 per window (interleaved after the last chunk's flush):
    TensorE transpose of agg_w, matmul with W (bf16), DVE bias+relu, DMA.
  - all scatter/matmul math in bf16 with fp32 PSUM accumulation
    (rel err ~4e-3, well within 2e-2).
"""
import math

import numpy as np
import ml_dtypes

import concourse.bacc as bacc
import concourse.mybir as mybir
import concourse.tile as tile
from concourse import bass_utils

P = 128
FEAT = 128
N_NODES = 100000
NCORES = 8
WINDOW = 128
CHUNK_ROWS = 25000
CALL_MAX_BLOCKS = 64

NPC = N_NODES // NCORES                   # 12500 dst nodes per core
NW = math.ceil(NPC / WINDOW)              # 98 windows per core
DPAD = NW * WINDOW                        # 12544 padded dst rows per core
NCHUNK = math.ceil(N_NODES / CHUNK_ROWS)  # 4 src chunks


def _bf16(a):
    return a.astype(ml_dtypes.bfloat16)


def _host_prep(x, edge_index, edge_weight, W, b):
    n = N_NODES
    src = np.asarray(edge_index[0], dtype=np.int64)
    dst = np.asarray(edge_index[1], dtype=np.int64)
    ew = np.asarray(edge_weight, dtype=np.float32)

    deg = np.bincount(dst, weights=ew.astype(np.float64), minlength=n)
    deg = (deg + 1.0).astype(np.float32)  # + self-loop weight
    dinv = np.where(deg > 0, 1.0 / np.sqrt(deg), 0.0).astype(np.float32)
    norm = (dinv[src] * ew * dinv[dst]).astype(np.float32)
    dinv2 = (dinv * dinv).astype(np.float32)

    core = dst // NPC
    dst_local = dst - core * NPC
    w_id = dst_local // WINDOW
    rel = (dst_local - w_id * WINDOW).astype(np.float32)
    c_id = src // CHUNK_ROWS
    idx_local = (src - c_id * CHUNK_ROWS).astype(np.int16)

    flat = (core * NCHUNK + c_id) * NW + w_id
    counts = np.bincount(flat, minlength=NCORES * NCHUNK * NW).reshape(
        NCORES, NCHUNK, NW)
    B = np.ceil(counts.max(axis=0) / P).astype(np.int64)

    nb_total = int(B.sum())
    slots_total = nb_total * P

    block_base = np.zeros((NCHUNK, NW), dtype=np.int64)
    regions = []
    acc = 0
    for c in range(NCHUNK):
        for w in range(NW):
            block_base[c, w] = acc
            regions.append((c, w, acc, int(B[c, w])))
            acc += int(B[c, w])

    calls = []
    for c in range(NCHUNK):
        b0 = int(block_base[c, 0])
        b1 = int(block_base[c + 1, 0]) if c + 1 < NCHUNK else nb_total
        k = b0
        while k < b1:
            nblk = min(CALL_MAX_BLOCKS, b1 - k)
            calls.append((c, k, nblk))
            k += nblk

    meta = dict(regions=regions, calls=calls, nb_total=nb_total,
                slots_total=slots_total, B=B)

    order_all = np.lexsort((w_id, c_id, core))
    core_sorted = core[order_all]
    core_starts = np.searchsorted(core_sorted, np.arange(NCORES + 1))

    x32 = np.ascontiguousarray(np.asarray(x, dtype=np.float32))
    xg = _bf16(x32)
    W16 = _bf16(np.ascontiguousarray(np.asarray(W, dtype=np.float32)))
    b32 = np.asarray(b, dtype=np.float32)
    btile = np.tile(b32[None, :], (P, 1)).astype(np.float32)
    cmp_t = np.tile(np.arange(WINDOW, dtype=np.float32)[None, :], (P, 1))
    ident = _bf16(np.eye(P, dtype=np.float32))

    in_maps = []
    for m in range(NCORES):
        sel = order_all[core_starts[m]:core_starts[m + 1]]
        midx, mrel, mnorm = idx_local[sel], rel[sel], norm[sel]

        rng = np.random.default_rng(12345 + m)
        idx16 = rng.integers(0, CHUNK_ROWS, slots_total).astype(np.int16)
        relq = np.full(slots_total, -1.0, dtype=np.float32)
        nrm = np.zeros(slots_total, dtype=np.float32)
        pos = 0
        for (c, w, blk0, nblk) in regions:
            cnt = int(counts[m, c, w])
            s0 = blk0 * P
            idx16[s0:s0 + cnt] = midx[pos:pos + cnt]
            relq[s0:s0 + cnt] = mrel[pos:pos + cnt]
            nrm[s0:s0 + cnt] = mnorm[pos:pos + cnt]
            pos += cnt
        assert pos == len(sel)

        idx_tile = np.zeros((P, slots_total // 16), dtype=np.int16)
        for (c, blk0, nblk) in calls:
            s0, s1 = blk0 * P, (blk0 + nblk) * P
            seg = idx16[s0:s1].reshape(-1, 16).T
            idx_tile[:, s0 // 16:s1 // 16] = np.tile(seg, (8, 1))

        xs = np.zeros((DPAD, FEAT), dtype=np.float32)
        xs[:NPC] = x32[m * NPC:(m + 1) * NPC]
        xsh = np.ascontiguousarray(
            xs.reshape(NW, P, FEAT).transpose(1, 0, 2).reshape(P, NW * FEAT))
        d2 = np.zeros((DPAD,), dtype=np.float32)
        d2[:NPC] = dinv2[m * NPC:(m + 1) * NPC]
        d2t = np.ascontiguousarray(d2.reshape(NW, P).T)

        in_maps.append({
            "xg": xg,
            "idx": idx_tile,
            "rel": relq.reshape(nb_total, P).T.copy(),
            "nrm": nrm.reshape(nb_total, P).T.copy(),
            "cmp": cmp_t,
            "xsh": _bf16(xsh),
            "d2t": d2t,
            "Wt": W16,
            "btile": btile,
            "ident": ident,
        })
    return meta, in_maps


def _build_kernel(meta):
    nb_total = meta["nb_total"]
    slots_total = meta["slots_total"]
    regions = meta["regions"]
    calls = meta["calls"]
    gdt = mybir.dt.bfloat16

    nc = bacc.Bacc("TRN2", target_bir_lowering=False, debug=False,
                   num_devices=NCORES)
    xg = nc.dram_tensor("xg", [N_NODES, FEAT], gdt, kind="ExternalInput")
    idx = nc.dram_tensor("idx", [P, slots_total // 16], mybir.dt.int16,
                         kind="ExternalInput")
    rel = nc.dram_tensor("rel", [P, nb_total], mybir.dt.float32,
                         kind="ExternalInput")
    nrm = nc.dram_tensor("nrm", [P, nb_total], mybir.dt.float32,
                         kind="ExternalInput")
    cmp_d = nc.dram_tensor("cmp", [P, WINDOW], mybir.dt.float32,
                           kind="ExternalInput")
    xsh = nc.dram_tensor("xsh", [P, NW * FEAT], mybir.dt.bfloat16,
                         kind="ExternalInput")
    d2t = nc.dram_tensor("d2t", [P, NW], mybir.dt.float32,
                         kind="ExternalInput")
    Wt = nc.dram_tensor("Wt", [FEAT, FEAT], mybir.dt.bfloat16,
                        kind="ExternalInput")
    btile = nc.dram_tensor("btile", [P, FEAT], mybir.dt.float32,
                           kind="ExternalInput")
    ident = nc.dram_tensor("ident", [P, P], mybir.dt.bfloat16,
                           kind="ExternalInput")
    out = nc.dram_tensor("out", [DPAD, FEAT], mybir.dt.float32,
                         kind="ExternalOutput")

    Bm = meta["B"]
    first_c = {}
    last_c = {}
    for w in range(NW):
        cs = [c for c in range(NCHUNK) if Bm[c, w] > 0]
        first_c[w] = cs[0] if cs else None
        last_c[w] = cs[-1] if cs else None

    with tile.TileContext(nc) as tc:
        with (
            tc.tile_pool(name="const", bufs=1) as constp,
            tc.tile_pool(name="agg", bufs=1) as aggp,
            tc.tile_pool(name="gbuf", bufs=3) as gbufp,
            tc.tile_pool(name="gsc", bufs=2) as gscp,
            tc.tile_pool(name="sel", bufs=8) as selp,
            tc.tile_pool(name="sl", bufs=4) as slp,
            tc.tile_pool(name="aT", bufs=3) as aTp,
            tc.tile_pool(name="ps1", bufs=2, space="PSUM") as ps1p,
            tc.tile_pool(name="tpg", bufs=6, space="PSUM") as tpgp,
            tc.tile_pool(name="outst", bufs=3) as outp,
        ):
            idx_sb = constp.tile([P, slots_total // 16], mybir.dt.int16)
            rel_sb = constp.tile([P, nb_total], mybir.dt.float32)
            nrm_sb = constp.tile([P, nb_total], mybir.dt.float32)
            cmp_sb = constp.tile([P, WINDOW], mybir.dt.float32)
            xsh_sb = constp.tile([P, NW * FEAT], mybir.dt.bfloat16)
            d2_sb = constp.tile([P, NW], mybir.dt.float32)
            W_sb = constp.tile([FEAT, FEAT], mybir.dt.bfloat16)
            b_sb = constp.tile([P, FEAT], mybir.dt.float32)
            id_sb = constp.tile([P, P], mybir.dt.bfloat16)
            agg = aggp.tile([P, NW * FEAT], mybir.dt.bfloat16)

            nc.sync.dma_start(out=idx_sb[:], in_=idx[:])
            nc.sync.dma_start(out=rel_sb[:], in_=rel[:])
            nc.sync.dma_start(out=nrm_sb[:], in_=nrm[:])
            nc.sync.dma_start(out=cmp_sb[:], in_=cmp_d[:])
            nc.sync.dma_start(out=xsh_sb[:], in_=xsh[:])
            nc.sync.dma_start(out=d2_sb[:], in_=d2t[:])
            nc.sync.dma_start(out=W_sb[:], in_=Wt[:])
            nc.sync.dma_start(out=b_sb[:], in_=btile[:])
            nc.sync.dma_start(out=id_sb[:], in_=ident[:])

            gtiles = {}
            issued = set()
            call_of_block = {}
            for ci, (c, blk0, nblk) in enumerate(calls):
                for bb in range(blk0, blk0 + nblk):
                    call_of_block[bb] = ci

            def gather_call(ci):
                c, blk0, nblk = calls[ci]
                nidx = nblk * P
                g = gbufp.tile([P, 1, CALL_MAX_BLOCKS * P], gdt, tag="g")
                nc.gpsimd.dma_gather(
                    g[:, :, :nidx],
                    xg[c * CHUNK_ROWS:min((c + 1) * CHUNK_ROWS, N_NODES), :],
                    idx_sb[:, blk0 * 8:(blk0 + nblk) * 8],
                    nidx, nidx, FEAT, transpose=True, single_packet=False,
                )
                # batched transpose-back into a per-call bf16 mega tile
                gsc_call = gscp.tile([P, CALL_MAX_BLOCKS, FEAT],
                                     mybir.dt.bfloat16, tag="gsc")
                for bb in range(nblk):
                    tpg = tpgp.tile([P, P], mybir.dt.bfloat16, tag="tpg")
                    nc.tensor.transpose(
                        tpg[:], g[:, 0, bb * P:(bb + 1) * P], id_sb[:])
                    nc.scalar.activation(
                        gsc_call[:, bb, :], tpg[:],
                        mybir.ActivationFunctionType.Copy)
                gtiles[ci] = (gsc_call, blk0, nblk)

            def phase2(w):
                tp = tpgp.tile([P, P], mybir.dt.bfloat16, tag="tpg")
                nc.tensor.transpose(
                    tp[:], agg[:, w * FEAT:(w + 1) * FEAT], id_sb[:])
                aT = aTp.tile([P, P], mybir.dt.bfloat16, tag="aT")
                nc.vector.tensor_copy(out=aT[:], in_=tp[:])
                ps2 = ps1p.tile([P, FEAT], mybir.dt.float32, tag="ps1")
                nc.tensor.matmul(out=ps2[:], lhsT=aT[:], rhs=W_sb[:],
                                 start=True, stop=True)
                ot = outp.tile([P, FEAT], mybir.dt.float32, tag="ot")
                nc.vector.tensor_add(out=ot[:], in0=ps2[:], in1=b_sb[:])
                nc.vector.tensor_scalar_max(ot[:], ot[:], 0.0)
                nc.sync.dma_start(out=out[w * P:(w + 1) * P, :], in_=ot[:])

            for (c, w, blk0, nblk) in regions:
                if nblk == 0:
                    if first_c[w] is None and c == 0:
                        sl = slp.tile([P, FEAT], mybir.dt.float32, tag="sl")
                        nc.scalar.activation(
                            sl[:], xsh_sb[:, w * FEAT:(w + 1) * FEAT],
                            mybir.ActivationFunctionType.Copy,
                            scale=d2_sb[:, w:w + 1])
                        nc.vector.tensor_copy(
                            out=agg[:, w * FEAT:(w + 1) * FEAT], in_=sl[:])
                        phase2(w)
                    continue
                ps = ps1p.tile([P, WINDOW], mybir.dt.float32, tag="ps1")
                for j, gb in enumerate(range(blk0, blk0 + nblk)):
                    ci = call_of_block[gb]
                    if ci not in issued:
                        gather_call(ci)
                        issued.add(ci)
                    g, cblk0, cnblk = gtiles[ci]
                    col = gb - cblk0
                    st = selp.tile([P, WINDOW], mybir.dt.bfloat16, tag="sel")
                    nc.vector.tensor_scalar(
                        out=st[:], in0=cmp_sb[:],
                        scalar1=rel_sb[:, gb:gb + 1],
                        scalar2=nrm_sb[:, gb:gb + 1],
                        op0=mybir.AluOpType.is_equal,
                        op1=mybir.AluOpType.mult,
                    )
                    nc.tensor.matmul(
                        out=ps[:], lhsT=st[:], rhs=g[:, col, :],
                        start=(j == 0), stop=(j == nblk - 1),
                    )
                wsl = agg[:, w * FEAT:(w + 1) * FEAT]
                if c == first_c[w]:
                    sl = slp.tile([P, FEAT], mybir.dt.float32, tag="sl")
                    nc.scalar.activation(
                        sl[:], xsh_sb[:, w * FEAT:(w + 1) * FEAT],
                        mybir.ActivationFunctionType.Copy,
                        scale=d2_sb[:, w:w + 1])
                    nc.vector.tensor_tensor(
                        out=wsl, in0=ps[:], in1=sl[:],
                        op=mybir.AluOpType.add)
                else:
                    nc.vector.tensor_add(out=wsl, in0=wsl, in1=ps[:])
                if c == last_c[w]:
                    phase2(w)
    nc.compile()
    return nc


def kernel(x, edge_index, edge_weight, W, b):
    assert x.shape == (N_NODES, FEAT)
    meta, in_maps = _host_prep(x, edge_index, edge_weight, W, b)
    nc = _build_kernel(meta)
    res = bass_utils.run_bass_kernel_spmd(
        nc, in_maps, core_ids=list(range(NCORES)), trace=False)
    outs = [res.results[m]["out"][:NPC] for m in range(NCORES)]
    return np.ascontiguousarray(np.concatenate(outs, axis=0))
